# revision 15
# baseline (speedup 1.0000x reference)
"""Euclidean contrastive loss on 8 Trainium2 NeuronCores (Bass/Tile).

Strategy (SPMD, one program for all 8 cores, per-core data differs):
  - Host: cast tokens to bf16, labels to bf16; for core c build inputs rotated
    by c*1024 rows so every core's "own" rows are rows 0..1023 of its copy.
    All slice offsets in the device program are then compile-time constants.
  - Device per core:
      norms (DVE fused square+rowsum, fp32 accum) -> rsqrt -> normalize rows
      -> bounce normalized bf16 rows to HBM -> 16 big DMA-xbar transposes into
      tT[k] = [128, 8192] (k = 4 K-tiles of the 512 feature dim).
      sim block = tT.T @ tT per (128-row block) x (2048-col group), PSUM fp32.
      Diagonal fix: sim[ii] -= 2 (one fused DVE op on the one 512-slice that
      contains the diagonal) so d2 stays positive and exp(diag) ~ 0.
      dist/tau = ACT Sqrt(scale*sim+bias) from PSUM -> fp16 tiles (phase A,
      sqrt table), masked-dist row sums + n_pos row sums via fused DVE ops.
      Phase B (exp/ln table): exp(-dist/tau) with free row-sum accumulation,
      then LSE = Ln(rowsum). Two A/B phase pairs -> only 4 ACT table loads.
      numer_i = n_pos_i * LSE_i + (masked dist sum_i - 2/tau).
  - Host: loss = sum(numer) / sum(n_pos) over all cores in float64.
"""

import os
import sys

import numpy as np
import ml_dtypes

try:
    import concourse.bass as bass  # noqa: F401
except ImportError:  # harness runs from a bare directory
    for p in ("/opt/trn_rl_repo", os.path.expanduser("~/.axon_site/_ro/trn_rl_repo")):
        if os.path.isdir(p) and p not in sys.path:
            sys.path.insert(0, p)
    import concourse.bass as bass  # noqa: F401

import concourse.mybir as mybir
import concourse.tile as tile
from concourse import bacc, bass_utils

N, D, NCORES = 8192, 512, 8
RPC = N // NCORES        # 1024 rows per core
NB = RPC // 128          # 8 row blocks of 128
KT = D // 128            # 4 contraction tiles
GW = 2048                # column group width
NG = N // GW             # 4 column groups
PH = 2                   # phase pairs (sqrt-batch then exp-batch), NB/PH*... blocks per phase
BPP = NB // PH           # blocks per phase pair (4)

BF16 = mybir.dt.bfloat16
FP16 = mybir.dt.float16
FP32 = mybir.dt.float32
AX = mybir.AxisListType.X
OP = mybir.AluOpType
AF = mybir.ActivationFunctionType

_CACHE: dict = {}
last_results = None  # test harness reads exec_time_ns from here


def _build(tau: float, stage: int = 5):
    nc = bacc.Bacc(
        "TRN2",
        target_bir_lowering=False,
        debug=False,
        enable_asserts=False,
        num_devices=NCORES,
    )
    tok = nc.dram_tensor("tok", [N, D], BF16, kind="ExternalInput")
    lab_bc = nc.dram_tensor("lab_bc", [128, N], BF16, kind="ExternalInput")
    lab_rows = nc.dram_tensor("lab_rows", [128, NB], FP32, kind="ExternalInput")
    out = nc.dram_tensor("part", [128, 2 * NB], FP32, kind="ExternalOutput")

    A = 2.0 / (tau * tau)  # dist/tau = sqrt(A - A*sim)

    # substage feature gates (stage 11/12/13/14 bisect stage 1)
    f_loads = stage != 11
    f_sqrt = stage not in (11, 12)
    f_norm = stage not in (11, 12, 13)
    f_store = stage not in (11, 12, 13, 14)
    f_transpose = 2 <= stage <= 5
    f_main = 3 <= stage <= 5
    f_fin = 4 <= stage <= 5

    with tile.TileContext(nc) as tc:
        with (
            tc.tile_pool(name="persist", bufs=1) as pp,
            tc.tile_pool(name="rows", bufs=16) as rows,
            tc.tile_pool(name="dist", bufs=BPP) as distp,
            tc.tile_pool(name="scratch", bufs=1) as sc,
            tc.tile_pool(name="psum", bufs=2, space="PSUM") as psum,
            tc.tile_pool(name="dram", bufs=1, space="DRAM") as dram,
        ):
            # ---- persistent tiles ----
            tT = [
                pp.tile([128, N], BF16, tag=f"tT{k}", name=f"tT{k}")
                for k in range(KT)
            ]
            Lc = pp.tile([128, N], BF16, tag="Lc")
            lr = pp.tile([128, NB], FP32, tag="lr")
            dms = pp.tile([128, 4 * 512], BF16, tag="dms")
            norm2 = pp.tile([128, 64], FP32, tag="norm2")
            nrm = pp.tile([128, 64], FP32, tag="nrm")
            inv = pp.tile([128, 64], FP32, tag="inv")
            esum = pp.tile([128, NB * NG], FP32, tag="esum")
            msac = pp.tile([128, NB * NG], FP32, tag="msac")
            npac = pp.tile([128, NB * NG], FP32, tag="npac")
            rowsum = pp.tile([128, NB], FP32, tag="rowsum")
            lse = pp.tile([128, NB], FP32, tag="lse")
            ms2 = pp.tile([128, NB], FP32, tag="ms2")
            np2 = pp.tile([128, NB], FP32, tag="np2")
            prod = pp.tile([128, NB], FP32, tag="prod")
            outp = pp.tile([128, 2 * NB], FP32, tag="outp")

            norm_hbm = dram.tile([N, D], BF16)

            biasA = pp.tile([128, 1], FP32, tag="biasA")
            nc.gpsimd.memset(biasA[:], float(A))

            # ---- labels ----
            nc.sync.dma_start(Lc[:], lab_bc[:, :])
            nc.sync.dma_start(lr[:], lab_rows[:, :])

            # ---- diagonal one-hot masks dm_k[p, f] = (f - p == 128k) ----
            iot = sc.tile([128, 512], mybir.dt.int32, tag="iot")
            nc.gpsimd.iota(iot[:], pattern=[[1, 512]], base=0, channel_multiplier=-1)
            iotf = sc.tile([128, 512], FP32, tag="iotf")
            nc.vector.tensor_copy(iotf[:], iot[:])
            for kk in range(4):
                nc.vector.tensor_scalar(
                    dms[:, kk * 512:(kk + 1) * 512], iotf[:],
                    float(kk * 128), None, op0=OP.is_equal,
                )

            # ---- load rows, norms, normalize, bounce to HBM ----
            junk = sc.tile([128, D], BF16, tag="junk")
            rowts = []
            for j in range(64 if f_loads else 0):
                rowt = rows.tile([128, D], BF16, tag="rowt")
                rowts.append(rowt)
                nc.sync.dma_start(rowt[:], tok[j * 128:(j + 1) * 128, :])
                nc.vector.scalar_tensor_tensor(
                    out=junk[:], in0=rowt[:], scalar=1.0, in1=rowt[:],
                    op0=OP.mult, op1=OP.mult, accum_out=norm2[:, j:j + 1],
                )
                if j % 8 == 7 and f_sqrt:
                    g8 = j // 8
                    s = slice(g8 * 8, g8 * 8 + 8)
                    nc.scalar.activation(nrm[:, s], norm2[:, s], AF.Sqrt)
                    nc.vector.reciprocal(inv[:, s], nrm[:, s])
                    for jj in range(g8 * 8, g8 * 8 + 8 if f_norm else g8 * 8):
                        rt = rowts[jj]
                        nc.vector.tensor_scalar(
                            rt[:], rt[:], inv[:, jj:jj + 1], None, op0=OP.mult,
                        )
                        if f_store:
                            nc.sync.dma_start(
                                norm_hbm[jj * 128:(jj + 1) * 128, :], rt[:],
                            )

            if stage == 11:
                nc.vector.tensor_copy(outp[:], dms[:, :16])
                nc.sync.dma_start(out[:, :], outp[:])
            if stage == 12:
                nc.vector.tensor_copy(outp[:], norm2[:, :16])
                nc.sync.dma_start(out[:, :], outp[:])
            if stage in (13, 14, 1):
                nc.vector.tensor_copy(outp[:], inv[:, :16])
                nc.sync.dma_start(out[:, :], outp[:])

            # ---- transpose to tT[k][:, j] via DMA xbar (HBM -> SBUF) ----
            if f_transpose:
                for k in range(KT):
                    for jg in range(4):
                        nc.sync.dma_start(
                            tT[k][:, jg * GW:(jg + 1) * GW],
                            norm_hbm[jg * GW:(jg + 1) * GW, k * 128:(k + 1) * 128],
                            transpose=True,
                        )

            if stage == 2:
                nc.vector.tensor_copy(outp[:], tT[0][:, :16])
                nc.sync.dma_start(out[:, :], outp[:])

            # ---- main compute: PH phase pairs over row blocks ----
            dist_of = {}
            for ph in range(PH if f_main else 0):
                blocks = range(ph * BPP, (ph + 1) * BPP)
                # phase A: matmuls + diag fix + sqrt (sqrt table) + DVE sums
                for m in blocks:
                    dist_m = distp.tile([128, N], FP16, tag="dist")
                    dist_of[m] = dist_m
                    lhsT = None
                    for g in range(NG):
                        ps = psum.tile([128, GW], FP32, tag="ps")
                        for k in range(KT):
                            lhsT = tT[k][:, m * 128:(m + 1) * 128]
                            for n in range(GW // 512):
                                nc.tensor.matmul(
                                    ps[:, n * 512:(n + 1) * 512],
                                    lhsT,
                                    tT[k][:, g * GW + n * 512: g * GW + (n + 1) * 512],
                                    start=(k == 0),
                                    stop=(k == KT - 1),
                                )
                        if g == 0:
                            nd = m // 4  # diag chunk within group 0
                            dsl = slice(nd * 512, (nd + 1) * 512)
                            nc.vector.scalar_tensor_tensor(
                                out=ps[:, dsl],
                                in0=dms[:, (m % 4) * 512:(m % 4 + 1) * 512],
                                scalar=-2.0,
                                in1=ps[:, dsl],
                                op0=OP.mult, op1=OP.add,
                            )
                        gs = slice(g * GW, (g + 1) * GW)
                        nc.scalar.activation(
                            dist_m[:, gs], ps[:], AF.Sqrt, bias=biasA[:],
                            scale=float(-A),
                        )
                        c = m * NG + g
                        stt_out = sc.tile([128, GW], FP16, tag="stt_out")
                        nc.vector.scalar_tensor_tensor(
                            out=stt_out[:], in0=Lc[:, gs], scalar=lr[:, m:m + 1],
                            in1=dist_m[:, gs], op0=OP.is_equal, op1=OP.mult,
                            accum_out=msac[:, c:c + 1],
                        )
                        np_out = sc.tile([128, GW], BF16, tag="np_out")
                        nc.vector.tensor_scalar(
                            np_out[:], Lc[:, gs], lr[:, m:m + 1], None,
                            op0=OP.is_equal, op1=OP.add,
                            accum_out=npac[:, c:c + 1],
                        )
                if stage == 3:
                    continue
                # phase B: exp with row-sum accumulation (exp/ln table)
                for m in blocks:
                    dist_m = dist_of[m]
                    for g in range(NG):
                        gs = slice(g * GW, (g + 1) * GW)
                        c = m * NG + g
                        exp_out = sc.tile([128, GW], FP16, tag="exp_out")
                        nc.scalar.activation(
                            exp_out[:], dist_m[:, gs], AF.Exp, scale=-1.0,
                            accum_out=esum[:, c:c + 1],
                        )
                for m in blocks:
                    nc.vector.reduce_sum(
                        rowsum[:, m:m + 1], esum[:, m * NG:(m + 1) * NG], axis=AX,
                    )
                bs = slice(ph * BPP, (ph + 1) * BPP)
                nc.scalar.activation(lse[:, bs], rowsum[:, bs], AF.Ln)

            if stage == 3:
                nc.vector.tensor_copy(outp[:], msac[:, :16])
                nc.sync.dma_start(out[:, :], outp[:])

            # ---- finalize ----
            for m in range(NB if f_fin else 0):
                nc.vector.reduce_sum(
                    ms2[:, m:m + 1], msac[:, m * NG:(m + 1) * NG], axis=AX,
                )
                nc.vector.reduce_sum(
                    np2[:, m:m + 1], npac[:, m * NG:(m + 1) * NG], axis=AX,
                )
                # npos output (col NB+m)
                nc.vector.tensor_scalar(
                    outp[:, NB + m:NB + m + 1], np2[:, m:m + 1], -1.0, None,
                    op0=OP.add,
                )
                # prod = (npos) * lse
                nc.vector.scalar_tensor_tensor(
                    out=prod[:, m:m + 1], in0=np2[:, m:m + 1], scalar=-1.0,
                    in1=lse[:, m:m + 1], op0=OP.add, op1=OP.mult,
                )
                # numer = (ms2 - 2/tau) + prod
                nc.vector.scalar_tensor_tensor(
                    out=outp[:, m:m + 1], in0=ms2[:, m:m + 1],
                    scalar=float(-2.0 / tau), in1=prod[:, m:m + 1],
                    op0=OP.add, op1=OP.add,
                )
            if f_fin:
                nc.sync.dma_start(out[:, :], outp[:])

    nc.compile()
    return nc


def _get_program(tau: float, stage: int = 5):
    key = (tau, stage)
    if key not in _CACHE:
        _CACHE[key] = _build(tau, stage)
    return _CACHE[key]


def make_in_maps(tokens: np.ndarray, labels: np.ndarray):
    bf = ml_dtypes.bfloat16
    tok_bf = np.asarray(tokens, dtype=np.float32).astype(bf)
    lab_f = np.asarray(labels).astype(np.float32).astype(bf)
    in_maps = []
    for c in range(NCORES):
        sh = c * RPC
        tok_rot = np.roll(tok_bf, -sh, axis=0)
        lab_rot = np.roll(lab_f, -sh)
        lab_bc = np.ascontiguousarray(np.broadcast_to(lab_rot[None, :], (128, N)))
        lab_rows = np.ascontiguousarray(
            lab_rot[: RPC].reshape(NB, 128).T.astype(np.float32)
        )  # [128, NB]
        in_maps.append({
            "tok": np.ascontiguousarray(tok_rot),
            "lab_bc": lab_bc,
            "lab_rows": lab_rows,
        })
    return in_maps


def _install_ntff_hook_shim():
    """Provide antenv.axon_hooks if the image lacks it (NTFF profiling via
    direct ctypes calls into libaxon_pjrt.so)."""
    try:
        from antenv.axon_hooks import get_axon_ntff_profile_hook  # noqa: F401
        return True
    except ImportError:
        pass
    so_path = "/opt/axon/libaxon_pjrt.so"
    if not os.path.exists(so_path):
        return False
    import contextlib
    import ctypes
    import types

    lib = ctypes.CDLL(so_path)
    if not hasattr(lib, "axon_start_nrt_profile"):
        return False
    lib.axon_start_nrt_profile.argtypes = [
        ctypes.POINTER(ctypes.c_int64), ctypes.c_size_t,
    ]
    lib.axon_start_nrt_profile.restype = ctypes.c_int64
    lib.axon_stop_nrt_profile.argtypes = [ctypes.c_char_p]
    lib.axon_stop_nrt_profile.restype = ctypes.c_int64

    @contextlib.contextmanager
    def _hook(output_dir, device_ids):
        import jax
        jax.devices()
        if device_ids:
            ids = (ctypes.c_int64 * len(device_ids))(*device_ids)
            rc = lib.axon_start_nrt_profile(ids, len(device_ids))
        else:
            rc = lib.axon_start_nrt_profile(None, 0)
        if rc != 0:
            raise RuntimeError(f"axon_start_nrt_profile rc={rc}")
        try:
            yield
        finally:
            n = lib.axon_stop_nrt_profile(str(output_dir).encode())
            if n < 0:
                raise RuntimeError(f"axon_stop_nrt_profile rc={n}")
            print(f"profile: {n} file(s) written to {output_dir}")

    mod = types.ModuleType("antenv.axon_hooks")
    mod.get_axon_ntff_profile_hook = lambda: _hook
    mod.set_axon_ntff_profile_hook = lambda h: None
    sys.modules["antenv.axon_hooks"] = mod
    return True


def kernel(tokens, labels, temperature=0.07):
    global last_results
    stage = int(os.environ.get("KBENCH_STAGE", "5"))
    nc = _get_program(float(temperature), stage)
    in_maps = make_in_maps(tokens, labels)
    trace = bool(int(os.environ.get("KBENCH_TRACE", "0")))
    if trace:
        trace = _install_ntff_hook_shim()
    res = bass_utils.run_bass_kernel_spmd(
        nc, in_maps, core_ids=list(range(NCORES)),
        trace=trace,
    )
    last_results = res
    num = 0.0
    den = 0.0
    for c in range(NCORES):
        p = res.results[c]["part"]
        num += p[:, :NB].astype(np.float64).sum()
        den += p[:, NB:].astype(np.float64).sum()
    return np.float32(num / den)
